# revision 50
# baseline (speedup 1.0000x reference)
"""Trainium2 Bass kernel for nn_CausalFieldAttention.

Shapes (hardcoded): B=4, N=4096, D=1024, H=16, hd=64, G=512, sigma=3.

Reference computation (the q-projection is computed but unused -> skipped):
    k  = x @ k_w.T + k_b                      (B,N,D) -> heads (B,H,N,hd)
    v  = x @ v_w.T + v_b
    wv = v * ||k||_head                       per-token, per-head scale
    field = segment_sum(wv, field_idx, G)     scatter tokens -> G bins
    conv  = circular_conv(field, causal_ker)  (reference: via rfft/irfft)
    y  = conv[field_idx]                      gather bins -> tokens
    out = y @ out_w.T + out_b
    (q/k/v/out biases are all zero in the reference inputs)

Device strategy: 8 cores = 4 batches x 2 head-groups (8 heads / 512 channels
each).  Precision plan (tolerance is 2e-2 rel-max; measured 6.5e-3 offline):
  - k projection: fp8 e4m3 DoubleRow matmuls (2 contraction rows/cycle).
    k only feeds ||k|| per head, which averages the fp8 error over 64
    channels -> 0.45% impact.  Weights are scaled by 2^7 host-side to stay
    in fp8 normal range; the scale is removed by folding 2^-7 into the
    conv matrix (exact power of two).
  - v projection: bf16 (fp8 here costs 3.6% error - too much).
  - everything else (scatter S, conv C, out_w, field/conv/A values, output
    partials) in bf16; PSUM accumulation is fp32 throughout.
  - scatter: block-sparse 0/1 matrix S (SBUF-resident; tokens sorted by
    bin so each 128-token tile hits 1-2 bin tiles => ~1.2 matmuls/tile).
  - circular conv: exact circulant matmul, produced transposed.
  - KEY reassociation: out = gather(conv) @ out_w = gather(conv @ out_w).
    A = conv @ ow is computed once at bin granularity (512 rows instead of
    4096), then the gather IS the final matmul: out(t,e) = S.T @ A.
  - out-projection partial per core over its 512 channels, stored bf16;
    host sums the two head-group partials per batch in f32 and adds out_b.
Schedule: k-projection runs one token tile ahead of v so the fp8 stream
fills the startup window while v weights are still in flight; S/ST are
bulk-loaded (chunked for subtile deps) instead of per-block DMAs, which
removes ~70 descriptor issues (~600ns engine time each).
"""

import os
import sys
from contextlib import ExitStack

import numpy as np
import ml_dtypes

for _p in ("/opt/trn_rl_repo", "/root/.axon_site/_ro/trn_rl_repo"):
    if os.path.isdir(_p) and _p not in sys.path:
        sys.path.append(_p)

import concourse.bacc as bacc
import concourse.mybir as mybir
import concourse.tile as tile
from concourse.bass_utils import run_bass_kernel_spmd

B, N, D = 4, 4096, 1024
H, HD, G = 16, 64, 512
SIGMA = 3.0
P = 128
KT = D // P          # 8 bf16 contraction tiles over D (v projection)
KT2 = D // (2 * P)   # 4 fp8 DoubleRow contraction tiles over D (k projection)
TT = N // P          # 32 token tiles
GT = G // P          # 4 bin tiles
CLOC = 512           # channels per core (8 heads)
HLOC = CLOC // HD    # 8 heads per core
ECH = D // 512       # 2 chunks of out-channels for 512-wide psum
NCORES = 8
WSCALE = 2.0 ** 7    # fp8 k-weight prescale, removed via conv matrix

F32 = mybir.dt.float32
BF16 = mybir.dt.bfloat16
FP8 = mybir.dt.float8e4
NP_BF16 = ml_dtypes.bfloat16
NP_FP8 = ml_dtypes.float8_e4m3
DR = mybir.MatmulPerfMode.DoubleRow

# set by test harness to capture a profile; kernel() stores results here
TRACE = False
LAST_RESULT = None


def _field_idx():
    # exactly mirrors the reference (fp32 div then mul, trunc, clip)
    pos = np.arange(N, dtype=np.float32) / np.float32(N - 1) * np.float32(G - 1)
    return np.clip(pos.astype(np.int32), 0, G - 1)


def _causal_kernel():
    i = np.arange(G)
    dist = np.abs(i - G // 2)
    ker = np.where(i >= G // 2, 0.0, np.exp(-dist / SIGMA)).astype(np.float32)
    ker = ker / (ker.sum() + 1e-8)
    return ker


def _plans():
    idx = _field_idx()
    ker = _causal_kernel()
    gg = (np.arange(G)[None, :] - np.arange(G)[:, None]) % G  # CT[g, g2] = ker[(g2-g)%G]
    CTm = ker[gg].astype(np.float32)

    Smat = np.zeros((N, G), np.float32)
    Smat[np.arange(N), idx] = 1.0
    STm = np.ascontiguousarray(Smat.T)

    tt_gts = [sorted(set((idx[t * P:(t + 1) * P] // P).tolist())) for t in range(TT)]
    contribs = {gt: [t for t in range(TT) if gt in tt_gts[t]] for gt in range(GT)}
    scatter_plan = [
        [(gt, t == contribs[gt][0], t == contribs[gt][-1]) for gt in tt_gts[t]]
        for t in range(TT)
    ]
    conv_blocks = [
        [gt for gt in range(GT)
         if np.abs(CTm[gt * P:(gt + 1) * P, gp * P:(gp + 1) * P]).max() > 1e-12]
        for gp in range(GT)
    ]
    return idx, CTm, Smat, STm, scatter_plan, tt_gts, conv_blocks


def _build_program(with_kb, with_vb, scatter_plan, tt_gts, conv_blocks):
    nc = bacc.Bacc("TRN2", target_bir_lowering=False, debug=False,
                   num_devices=NCORES)
    # host-pre-arranged layouts: leading dim is the SBUF partition, the rest
    # is contiguous per partition so every DMA moves 1-16KB runs.
    x8d = nc.dram_tensor("x8", [P, TT, KT2 * 2 * P], FP8, kind="ExternalInput").ap()
    xbd = nc.dram_tensor("xb", [P, TT, KT * P], BF16, kind="ExternalInput").ap()
    kwd = nc.dram_tensor("kw8", [P, KT2, 2 * CLOC], FP8, kind="ExternalInput").ap()
    vwd = nc.dram_tensor("vw", [P, KT, CLOC], BF16, kind="ExternalInput").ap()
    owd = nc.dram_tensor("ow", [P, GT, D], BF16, kind="ExternalInput").ap()
    ctd = nc.dram_tensor("ct", [P, GT, G], BF16, kind="ExternalInput").ap()
    Sd = nc.dram_tensor("Smat", [P, TT, G], BF16, kind="ExternalInput").ap()
    STd = nc.dram_tensor("STm", [P, GT, N], BF16, kind="ExternalInput").ap()
    kb = nc.dram_tensor("kb", [1, CLOC], F32, kind="ExternalInput").ap() if with_kb else None
    vb = nc.dram_tensor("vb", [1, CLOC], F32, kind="ExternalInput").ap() if with_vb else None
    out_d = nc.dram_tensor("out", [N, D], BF16, kind="ExternalOutput").ap()

    with tile.TileContext(nc) as tc, ExitStack() as es:
        cpool = es.enter_context(tc.tile_pool(name="const", bufs=1))

        # resident tensors; k/v weights split per-kt so the first projection
        # matmuls only wait on their own slice (subtile deps).
        kw_sb = cpool.tile([P, KT2, 2, CLOC], FP8)
        vw_sb = cpool.tile([P, KT, CLOC], BF16)
        ow_sb = cpool.tile([P, GT, D], BF16)
        ct_sb = cpool.tile([P, GT, G], BF16)
        S_sb = cpool.tile([P, TT, G], BF16)
        ST_sb = cpool.tile([P, GT, N], BF16)
        field_sb = cpool.tile([P, GT, G], BF16)
        convT_sb = cpool.tile([P, GT, G], BF16)
        A_sb = cpool.tile([P, GT, D], BF16)
        if with_kb:
            kb_sb = cpool.tile([1, CLOC], F32)
            nc.sync.dma_start(kb_sb[:], kb[:])
        if with_vb:
            vb_sb = cpool.tile([1, CLOC], F32)
            nc.sync.dma_start(vb_sb[:], vb[:])

        opool = es.enter_context(tc.tile_pool(name="osb", bufs=6))

        # ---- phase 1: projections, ||k||, wv, scatter ----
        ph1 = ExitStack()
        xpool = ph1.enter_context(tc.tile_pool(name="xin", bufs=3))
        wvpool = ph1.enter_context(tc.tile_pool(name="wv", bufs=4))
        smpool = ph1.enter_context(tc.tile_pool(name="small", bufs=3))
        ps_k = ph1.enter_context(tc.tile_pool(name="ps_k", bufs=2, space="PSUM"))
        ps_v = ph1.enter_context(tc.tile_pool(name="ps_v", bufs=2, space="PSUM"))
        ps_f = ph1.enter_context(tc.tile_pool(name="ps_f", bufs=2, space="PSUM"))
        ps_mid = ph1.enter_context(tc.tile_pool(name="ps_mid", bufs=2, space="PSUM"))

        field_ps = {}
        # psum -> sbuf copies can only run on DVE or ACT (GpSimd has no PSUM
        # access); alternate between the two.
        cp_rot = [nc.vector.tensor_copy, nc.scalar.copy, nc.vector.tensor_copy]
        dma_rot = [nc.sync, nc.scalar, nc.gpsimd]

        # ---- mid-stage jobs: convT column-tiles, A slices, and token-tile
        # output writes, emitted inside phase 1 as their field deps complete.
        # conv_blocks[gp] lists the only bin-tiles feeding convT[:, gp] (the
        # causal kernel's support), so gp=2 is ready after field gt<=1, gp=3
        # after gt<=2; gp=0,1 wrap circularly and must wait for the end.
        def job_convT(gp, pool, tag):
            def run():
                mt = pool.tile([P, G], F32, tag=tag, name=f"cvt{gp}")
                blocks = conv_blocks[gp]
                for ct in range(GT):
                    for gi, gt in enumerate(blocks):
                        nc.tensor.matmul(
                            mt[:, ct * P:(ct + 1) * P],
                            field_sb[:, gt, ct * P:(ct + 1) * P],
                            ct_sb[:, gt, gp * P:(gp + 1) * P],
                            start=(gi == 0), stop=(gi == len(blocks) - 1))
                cp_rot[gp % 3](
                    convT_sb[:, :, gp * P:(gp + 1) * P],
                    mt[:].rearrange("p (ct f) -> p ct f", f=P))
            return run

        def job_A(gp, pool, tag):
            def run():
                for ec in range(ECH):
                    esl = slice(ec * 512, (ec + 1) * 512)
                    mt = pool.tile([P, 512], F32, tag=tag, name=f"amt{gp}_{ec}")
                    for ct in range(GT):
                        nc.tensor.matmul(mt[:],
                                         convT_sb[:, ct, gp * P:(gp + 1) * P],
                                         ow_sb[:, ct, esl],
                                         start=(ct == 0), stop=(ct == GT - 1))
                    cp_rot[(gp + ec + 1) % 3](A_sb[:, gp, esl], mt[:])
            return run

        def job_out(tt, pool, tag, split_store=False):
            def run():
                tsl = slice(tt * P, (tt + 1) * P)
                gts = tt_gts[tt]
                osb = opool.tile([P, D], BF16, tag="osb")
                for ec in range(ECH):
                    esl = slice(ec * 512, (ec + 1) * 512)
                    mt = pool.tile([P, 512], F32, tag=tag, name=f"omt{tt}_{ec}")
                    for i, gt in enumerate(gts):
                        nc.tensor.matmul(mt[:],
                                         ST_sb[:, gt, tsl],
                                         A_sb[:, gt, esl],
                                         start=(i == 0), stop=(i == len(gts) - 1))
                    cp_rot[(tt + ec) % 3](osb[:, esl], mt[:])
                    if split_store:
                        dma_rot[(tt + ec) % 3].dma_start(out_d[tsl, esl],
                                                         osb[:, esl])
                if not split_store:
                    dma_rot[tt % 3].dma_start(out_d[tsl, :], osb[:])
            return run

        # enqueue points: field copy for gt lands during iteration
        # (last_contrib(gt) + 1) via the pending-scatter delay
        last_tt = {gt: max(t for t in range(TT) if gt in tt_gts[t])
                   for gt in range(GT)}
        enqueue_at = {}
        ready2 = last_tt[1] + 2      # field gt0,gt1 copied
        ready3 = last_tt[2] + 2
        enqueue_at.setdefault(ready2, []).append(("cvt", 2))
        enqueue_at.setdefault(ready2 + 1, []).append(("A", 2))
        enqueue_at.setdefault(ready3, []).append(("cvt", 3))
        enqueue_at.setdefault(ready3 + 1, []).append(("A", 3))
        post_loop_outs = []
        for t in range(TT):
            if set(tt_gts[t]) <= {2}:
                enqueue_at.setdefault(ready2 + 2, []).append(("out", t))
            elif set(tt_gts[t]) <= {2, 3}:
                if len(post_loop_outs) < 2 and t >= TT - 3:
                    # held back: fills the PE while the last tile's wv chain
                    # (ACT/DVE) and field copy run before the ph2 tail
                    post_loop_outs.append(t)
                else:
                    enqueue_at.setdefault(ready3 + 2, []).append(("out", t))
        mid_queue = []

        fcopy = {0: nc.vector.tensor_copy, 1: nc.scalar.copy,
                 2: nc.vector.tensor_copy, 3: nc.scalar.copy}

        def emit_scatter(tt, wv):
            for gt, first, last in scatter_plan[tt]:
                if first:
                    field_ps[gt] = ps_f.tile([P, CLOC], F32, tag="fld",
                                             name=f"fld{gt}")
                nc.tensor.matmul(field_ps[gt][:],
                                 S_sb[:, tt, gt * P:(gt + 1) * P], wv[:],
                                 start=first, stop=last)
                if last:
                    fcopy[gt](field_sb[:, gt, :], field_ps[gt][:])

        # x tile management: k-projection runs one tile ahead of v.
        x8_tiles, xb_tiles = {}, {}

        def ensure_x8(tt, eng):
            if tt not in x8_tiles and tt < TT:
                x8_tiles[tt] = xpool.tile([P, KT2, 2, P], FP8, tag="x8blk",
                                          bufs=5, name=f"x8b{tt}")
                eng.dma_start(x8_tiles[tt][:], x8d[:, tt, :])

        def ensure_xb(tt, eng):
            if tt not in xb_tiles and tt < TT:
                xb_tiles[tt] = xpool.tile([P, KT, P], BF16, tag="xbblk",
                                          bufs=5, name=f"xbb{tt}")
                eng.dma_start(xb_tiles[tt][:], xbd[:, tt, :])

        # startup: weight loads batched into halves, ordered by first use
        # across the three DMA queues (SP / ACT / GpSimd).
        ensure_x8(0, nc.sync)
        nc.scalar.dma_start(kw_sb[:, 0:2, :, :], kwd[:, 0:2, :])
        ensure_x8(1, nc.gpsimd)
        ensure_xb(0, nc.sync)
        nc.scalar.dma_start(kw_sb[:, 2:4, :, :], kwd[:, 2:4, :])
        ensure_x8(2, nc.gpsimd)
        nc.scalar.dma_start(vw_sb[:, 0:4, :], vwd[:, 0:4, :])
        nc.sync.dma_start(vw_sb[:, 4:8, :], vwd[:, 4:8, :])
        for tt in range(1, 5):
            ensure_xb(tt, nc.sync)
        for tt in range(3, 5):
            ensure_x8(tt, nc.gpsimd)
        nc.gpsimd.dma_start(S_sb[:, 0:8, :], Sd[:, 0:8, :])

        # bulk constant loads, ordered by first use. Only the S chunks stay
        # on gpsimd (they feed the scatter); everything else rides the
        # scalar queue, which is idle mid-loop, so the steady x8 prefetches
        # never queue behind a 1MB constant (measured: k-proj LDWEIGHTS
        # waits of 3.4-5us on x8 semaphores at iterations 9-11 otherwise).
        gp_loads = {
            2: [(nc.gpsimd, S_sb[:, 8:16, :], Sd[:, 8:16, :])],
            4: [(nc.scalar, ST_sb[:, 2, :], STd[:, 2, :])],
            6: [(nc.scalar, ST_sb[:, 3, :], STd[:, 3, :])],
            8: [(nc.scalar, ct_sb[:], ctd[:]), (nc.scalar, ow_sb[:], owd[:])],
            10: [(nc.gpsimd, S_sb[:, 16:24, :], Sd[:, 16:24, :])],
            12: [(nc.scalar, ST_sb[:, 0, :], STd[:, 0, :])],
            14: [(nc.scalar, ST_sb[:, 1, :], STd[:, 1, :])],
            16: [(nc.gpsimd, S_sb[:, 24:32, :], Sd[:, 24:32, :])],
        }

        kps_t = {}
        km_t = {}
        KLAG = 1        # k-projection runs one tile ahead of v

        def emit_k(tt):
            kps_t[tt] = ps_k.tile([P, CLOC], F32, tag="kps", name=f"kps{tt}")
            for kt in range(KT2):
                nc.tensor.matmul(kps_t[tt][:], x8_tiles[tt][:, kt, :, :],
                                 kw_sb[:, kt, :, :],
                                 start=(kt == 0), stop=(kt == KT2 - 1),
                                 perf_mode=DR)

        def emit_km(tt):
            # ||k|| per head right after the k matmuls: frees the kps psum
            # quickly and precomputes km so wv is a single DVE op after v
            # (kps holds 2^7*k; the scale is removed via the conv matrix)
            kps = kps_t.pop(tt)
            if with_kb:
                nc.vector.tensor_tensor(
                    kps[:], kps[:], kb_sb[:].broadcast_to((P, CLOC)),
                    mybir.AluOpType.add)
            ksq = smpool.tile([P, CLOC], F32, tag="ksq", bufs=3,
                              name=f"ksq{tt}")
            nc.scalar.activation(ksq[:], kps[:], mybir.ActivationFunctionType.Square)
            km2 = smpool.tile([P, HLOC], F32, tag="km2", bufs=3,
                              name=f"km2{tt}")
            nc.vector.reduce_sum(km2[:], ksq[:].rearrange("p (h d) -> p h d", d=HD),
                                 axis=mybir.AxisListType.X)
            km_t[tt] = smpool.tile([P, HLOC], F32, tag="km", bufs=KLAG + 3,
                                   name=f"km{tt}")
            nc.scalar.sqrt(km_t[tt][:], km2[:])

        for tt in range(KLAG):      # prologue: pure-k warmup
            emit_k(tt)
            emit_km(tt)
        pending = None
        for tt in range(TT):
            ensure_x8(tt + KLAG + 3, nc.gpsimd)
            ensure_xb(tt + 3, nc.sync)
            for eng, dst, src in gp_loads.get(tt, []):
                eng.dma_start(dst, src)

            if tt + KLAG < TT:
                emit_k(tt + KLAG)
                emit_km(tt + KLAG)
            vps = ps_v.tile([P, CLOC], F32, tag="vps")
            korder = [4, 5, 6, 7, 0, 1, 2, 3] if tt < 2 else list(range(KT))
            for i, kt in enumerate(korder):
                nc.tensor.matmul(vps[:], xb_tiles[tt][:, kt, :], vw_sb[:, kt, :],
                                 start=(i == 0), stop=(i == KT - 1))
            if pending is not None:
                emit_scatter(*pending)
                pending = None
            if with_vb:
                nc.vector.tensor_tensor(
                    vps[:], vps[:], vb_sb[:].broadcast_to((P, CLOC)),
                    mybir.AluOpType.add)

            for kind, arg in enqueue_at.get(tt, []):
                mid_queue.append((kind, arg))
            for _ in range(2):
                if mid_queue:
                    kind, arg = mid_queue.pop(0)
                    mk = {"cvt": job_convT, "A": job_A, "out": job_out}[kind]
                    mk(arg, ps_mid, "mid")()

            # wv = v * ||k||, one DVE op via stride-0 broadcast of km
            km = km_t.pop(tt)
            wv = wvpool.tile([P, CLOC], BF16, tag="wv")
            nc.vector.tensor_tensor(
                wv[:].rearrange("p (h d) -> p h d", d=HD),
                vps[:].rearrange("p (h d) -> p h d", d=HD),
                km[:].unsqueeze(2).broadcast_to((P, HLOC, HD)),
                mybir.AluOpType.mult)
            pending = (tt, wv)

        emit_scatter(*pending)
        for t in post_loop_outs:
            job_out(t, ps_mid, "mid")()
        ph1.close()

        # flush any queued mid jobs (still inside ph1 pools)
        while mid_queue:
            kind, arg = mid_queue.pop(0)
            mk = {"cvt": job_convT, "A": job_A, "out": job_out}[kind]
            mk(arg, ps_mid, "mid")()

        # ---- tail: circular-wrap convT tiles 0,1 -> A -> remaining tokens ----
        ph2 = ExitStack()
        ps_t = ph2.enter_context(tc.tile_pool(name="ps_t", bufs=8, space="PSUM"))
        done = {t for jobs in enqueue_at.values() for k, t in jobs if k == "out"}
        done |= set(post_loop_outs)
        job_convT(0, ps_t, "tmid")()
        job_convT(1, ps_t, "tmid")()
        job_A(0, ps_t, "tmid")()
        job_A(1, ps_t, "tmid")()
        for t in range(TT):
            if t not in done:
                job_out(t, ps_t, "tmid")()
        ph2.close()

    nc.compile()
    return nc


_PROGRAM_CACHE = {}


def _get_program(with_kb, with_vb):
    key = (with_kb, with_vb)
    if key not in _PROGRAM_CACHE:
        _, _, _, _, sp, tg, cb = _plans()
        _PROGRAM_CACHE[key] = _build_program(with_kb, with_vb, sp, tg, cb)
    return _PROGRAM_CACHE[key]


def kernel(x, q_w, q_b, k_w, k_b, v_w, v_b, out_w, out_b):
    global LAST_RESULT
    x = np.asarray(x, dtype=np.float32)
    k_w = np.asarray(k_w, dtype=np.float32)
    k_b = np.asarray(k_b, dtype=np.float32)
    v_w = np.asarray(v_w, dtype=np.float32)
    v_b = np.asarray(v_b, dtype=np.float32)
    out_w = np.asarray(out_w, dtype=np.float32)
    out_b = np.asarray(out_b, dtype=np.float32)

    with_kb = bool(np.any(k_b))
    with_vb = bool(np.any(v_b))
    nc = _get_program(with_kb, with_vb)
    _, CTm, Smat, STm, _, _, _ = _plans()

    S_host = np.ascontiguousarray(
        Smat.reshape(TT, P, G).transpose(1, 0, 2)).astype(NP_BF16)
    ST_host = np.ascontiguousarray(
        STm.reshape(GT, P, N).transpose(1, 0, 2)).astype(NP_BF16)
    ct_host = np.ascontiguousarray(
        (CTm / WSCALE).astype(NP_BF16).reshape(GT, P, G).transpose(1, 0, 2))

    # per-batch x layouts (shared by both head-group cores of the batch)
    x8_host, xb_host = [], []
    for b in range(B):
        xt = x[b]                                   # (N, D)
        a8 = xt.reshape(TT, P, KT2, 2, P).transpose(4, 0, 2, 3, 1)
        x8_host.append(np.ascontiguousarray(a8).astype(NP_FP8).reshape(P, TT, KT2 * 2 * P))
        ab = xt.reshape(TT, P, KT, P).transpose(3, 0, 2, 1)
        xb_host.append(np.ascontiguousarray(ab).astype(NP_BF16).reshape(P, TT, KT * P))

    in_maps = []
    for c in range(NCORES):
        b, hg = c // 2, c % 2
        chs = slice(hg * CLOC, (hg + 1) * CLOC)
        kwT = np.ascontiguousarray(k_w[chs, :].T) * np.float32(WSCALE)  # (D, CLOC)
        vwT = np.ascontiguousarray(v_w[chs, :].T)
        owT = np.ascontiguousarray(out_w[:, chs].T)                     # (CLOC, D)
        m = {
            "x8": x8_host[b],
            "xb": xb_host[b],
            "kw8": np.ascontiguousarray(
                kwT.reshape(KT2, 2, P, CLOC).transpose(2, 0, 1, 3)
            ).astype(NP_FP8).reshape(P, KT2, 2 * CLOC),
            "vw": np.ascontiguousarray(
                vwT.reshape(KT, P, CLOC).transpose(1, 0, 2)).astype(NP_BF16),
            "ow": np.ascontiguousarray(
                owT.reshape(GT, P, D).transpose(1, 0, 2)).astype(NP_BF16),
            "ct": ct_host,
            "Smat": S_host,
            "STm": ST_host,
        }
        if with_kb:
            m["kb"] = np.ascontiguousarray(
                k_b[chs][None, :] * np.float32(WSCALE)).astype(np.float32)
        if with_vb:
            m["vb"] = np.ascontiguousarray(v_b[chs][None, :]).astype(np.float32)
        in_maps.append(m)

    res = run_bass_kernel_spmd(nc, in_maps, core_ids=list(range(NCORES)),
                               trace=TRACE)
    LAST_RESULT = res

    out = np.empty((B, N, D), dtype=np.float32)
    for b in range(B):
        out[b] = res.results[2 * b]["out"].astype(np.float32)
        out[b] += res.results[2 * b + 1]["out"].astype(np.float32)
        out[b] += out_b[None, :]
    return out


# revision 52
# speedup vs baseline: 1.0201x; 1.0201x over previous
"""Trainium2 Bass kernel for nn_CausalFieldAttention.

Shapes (hardcoded): B=4, N=4096, D=1024, H=16, hd=64, G=512, sigma=3.

Reference computation (the q-projection is computed but unused -> skipped):
    k  = x @ k_w.T + k_b                      (B,N,D) -> heads (B,H,N,hd)
    v  = x @ v_w.T + v_b
    wv = v * ||k||_head                       per-token, per-head scale
    field = segment_sum(wv, field_idx, G)     scatter tokens -> G bins
    conv  = circular_conv(field, causal_ker)  (reference: via rfft/irfft)
    y  = conv[field_idx]                      gather bins -> tokens
    out = y @ out_w.T + out_b
    (q/k/v/out biases are all zero in the reference inputs)

Device strategy: 8 cores = 4 batches x 2 head-groups (8 heads / 512 channels
each).  Precision plan (tolerance is 2e-2 rel-max; measured 6.5e-3 offline):
  - k projection: fp8 e4m3 DoubleRow matmuls (2 contraction rows/cycle).
    k only feeds ||k|| per head, which averages the fp8 error over 64
    channels -> 0.45% impact.  Weights are scaled by 2^7 host-side to stay
    in fp8 normal range; the scale is removed by folding 2^-7 into the
    conv matrix (exact power of two).
  - v projection: bf16 (fp8 here costs 3.6% error - too much).
  - everything else (scatter S, conv C, out_w, field/conv/A values, output
    partials) in bf16; PSUM accumulation is fp32 throughout.
  - scatter: block-sparse 0/1 matrix S (SBUF-resident; tokens sorted by
    bin so each 128-token tile hits 1-2 bin tiles => ~1.2 matmuls/tile).
  - circular conv: exact circulant matmul, produced transposed.
  - KEY reassociation: out = gather(conv) @ out_w = gather(conv @ out_w).
    A = conv @ ow is computed once at bin granularity (512 rows instead of
    4096), then the gather IS the final matmul: out(t,e) = S.T @ A.
  - out-projection partial per core over its 512 channels, stored bf16;
    host sums the two head-group partials per batch in f32 and adds out_b.
Schedule: k-projection runs one token tile ahead of v so the fp8 stream
fills the startup window while v weights are still in flight; S/ST are
bulk-loaded (chunked for subtile deps) instead of per-block DMAs, which
removes ~70 descriptor issues (~600ns engine time each).
"""

import os
import sys
from contextlib import ExitStack

import numpy as np
import ml_dtypes

for _p in ("/opt/trn_rl_repo", "/root/.axon_site/_ro/trn_rl_repo"):
    if os.path.isdir(_p) and _p not in sys.path:
        sys.path.append(_p)

import concourse.bacc as bacc
import concourse.mybir as mybir
import concourse.tile as tile
from concourse.bass_utils import run_bass_kernel_spmd

B, N, D = 4, 4096, 1024
H, HD, G = 16, 64, 512
SIGMA = 3.0
P = 128
KT = D // P          # 8 bf16 contraction tiles over D (v projection)
KT2 = D // (2 * P)   # 4 fp8 DoubleRow contraction tiles over D (k projection)
TT = N // P          # 32 token tiles
GT = G // P          # 4 bin tiles
CLOC = 512           # channels per core (8 heads)
HLOC = CLOC // HD    # 8 heads per core
ECH = D // 512       # 2 chunks of out-channels for 512-wide psum
NCORES = 8
WSCALE = 2.0 ** 7    # fp8 k-weight prescale, removed via conv matrix

F32 = mybir.dt.float32
BF16 = mybir.dt.bfloat16
FP8 = mybir.dt.float8e4
NP_BF16 = ml_dtypes.bfloat16
NP_FP8 = ml_dtypes.float8_e4m3
DR = mybir.MatmulPerfMode.DoubleRow

# set by test harness to capture a profile; kernel() stores results here
TRACE = False
LAST_RESULT = None


def _field_idx():
    # exactly mirrors the reference (fp32 div then mul, trunc, clip)
    pos = np.arange(N, dtype=np.float32) / np.float32(N - 1) * np.float32(G - 1)
    return np.clip(pos.astype(np.int32), 0, G - 1)


def _causal_kernel():
    i = np.arange(G)
    dist = np.abs(i - G // 2)
    ker = np.where(i >= G // 2, 0.0, np.exp(-dist / SIGMA)).astype(np.float32)
    ker = ker / (ker.sum() + 1e-8)
    return ker


def _plans():
    idx = _field_idx()
    ker = _causal_kernel()
    gg = (np.arange(G)[None, :] - np.arange(G)[:, None]) % G  # CT[g, g2] = ker[(g2-g)%G]
    CTm = ker[gg].astype(np.float32)

    Smat = np.zeros((N, G), np.float32)
    Smat[np.arange(N), idx] = 1.0
    STm = np.ascontiguousarray(Smat.T)

    tt_gts = [sorted(set((idx[t * P:(t + 1) * P] // P).tolist())) for t in range(TT)]
    contribs = {gt: [t for t in range(TT) if gt in tt_gts[t]] for gt in range(GT)}
    scatter_plan = [
        [(gt, t == contribs[gt][0], t == contribs[gt][-1]) for gt in tt_gts[t]]
        for t in range(TT)
    ]
    conv_blocks = [
        [gt for gt in range(GT)
         if np.abs(CTm[gt * P:(gt + 1) * P, gp * P:(gp + 1) * P]).max() > 1e-12]
        for gp in range(GT)
    ]
    return idx, CTm, Smat, STm, scatter_plan, tt_gts, conv_blocks


def _build_program(with_kb, with_vb, scatter_plan, tt_gts, conv_blocks):
    nc = bacc.Bacc("TRN2", target_bir_lowering=False, debug=False,
                   num_devices=NCORES)
    # host-pre-arranged layouts: leading dim is the SBUF partition, the rest
    # is contiguous per partition so every DMA moves 1-16KB runs.
    x8d = nc.dram_tensor("x8", [P, TT, KT2 * 2 * P], FP8, kind="ExternalInput").ap()
    xbd = nc.dram_tensor("xb", [P, TT, KT * P], BF16, kind="ExternalInput").ap()
    kwd = nc.dram_tensor("kw8", [P, KT2, 2 * CLOC], FP8, kind="ExternalInput").ap()
    vwd = nc.dram_tensor("vw", [P, KT, CLOC], BF16, kind="ExternalInput").ap()
    owd = nc.dram_tensor("ow", [P, GT, D], BF16, kind="ExternalInput").ap()
    ctd = nc.dram_tensor("ct", [P, GT, G], BF16, kind="ExternalInput").ap()
    Sd = nc.dram_tensor("Smat", [P, TT, G], BF16, kind="ExternalInput").ap()
    STd = nc.dram_tensor("STm", [P, GT, N], BF16, kind="ExternalInput").ap()
    kb = nc.dram_tensor("kb", [1, CLOC], F32, kind="ExternalInput").ap() if with_kb else None
    vb = nc.dram_tensor("vb", [1, CLOC], F32, kind="ExternalInput").ap() if with_vb else None
    out_d = nc.dram_tensor("out", [N, D], BF16, kind="ExternalOutput").ap()

    with tile.TileContext(nc) as tc, ExitStack() as es:
        cpool = es.enter_context(tc.tile_pool(name="const", bufs=1))

        # resident tensors; k/v weights split per-kt so the first projection
        # matmuls only wait on their own slice (subtile deps).
        kw_sb = cpool.tile([P, KT2, 2, CLOC], FP8)
        vw_sb = cpool.tile([P, KT, CLOC], BF16)
        ow_sb = cpool.tile([P, GT, D], BF16)
        ct_sb = cpool.tile([P, GT, G], BF16)
        S_sb = cpool.tile([P, TT, G], BF16)
        ST_sb = cpool.tile([P, GT, N], BF16)
        field_sb = cpool.tile([P, GT, G], BF16)
        convT_sb = cpool.tile([P, GT, G], BF16)
        A_sb = cpool.tile([P, GT, D], BF16)
        if with_kb:
            kb_sb = cpool.tile([1, CLOC], F32)
            nc.sync.dma_start(kb_sb[:], kb[:])
        if with_vb:
            vb_sb = cpool.tile([1, CLOC], F32)
            nc.sync.dma_start(vb_sb[:], vb[:])

        opool = es.enter_context(tc.tile_pool(name="osb", bufs=6))

        # ---- phase 1: projections, ||k||, wv, scatter ----
        ph1 = ExitStack()
        xpool = ph1.enter_context(tc.tile_pool(name="xin", bufs=3))
        wvpool = ph1.enter_context(tc.tile_pool(name="wv", bufs=4))
        smpool = ph1.enter_context(tc.tile_pool(name="small", bufs=3))
        ps_k = ph1.enter_context(tc.tile_pool(name="ps_k", bufs=2, space="PSUM"))
        ps_v = ph1.enter_context(tc.tile_pool(name="ps_v", bufs=2, space="PSUM"))
        ps_f = ph1.enter_context(tc.tile_pool(name="ps_f", bufs=2, space="PSUM"))
        ps_mid = ph1.enter_context(tc.tile_pool(name="ps_mid", bufs=2, space="PSUM"))

        field_ps = {}
        # psum -> sbuf copies can only run on DVE or ACT (GpSimd has no PSUM
        # access); alternate between the two.
        cp_rot = [nc.vector.tensor_copy, nc.scalar.copy, nc.vector.tensor_copy]
        dma_rot = [nc.sync, nc.scalar, nc.gpsimd]

        # ---- mid-stage jobs: convT column-tiles, A slices, and token-tile
        # output writes, emitted inside phase 1 as their field deps complete.
        # conv_blocks[gp] lists the only bin-tiles feeding convT[:, gp] (the
        # causal kernel's support), so gp=2 is ready after field gt<=1, gp=3
        # after gt<=2; gp=0,1 wrap circularly and must wait for the end.
        def job_convT(gp, pool, tag):
            def run():
                mt = pool.tile([P, G], F32, tag=tag, name=f"cvt{gp}")
                blocks = conv_blocks[gp]
                for ct in range(GT):
                    for gi, gt in enumerate(blocks):
                        nc.tensor.matmul(
                            mt[:, ct * P:(ct + 1) * P],
                            field_sb[:, gt, ct * P:(ct + 1) * P],
                            ct_sb[:, gt, gp * P:(gp + 1) * P],
                            start=(gi == 0), stop=(gi == len(blocks) - 1))
                cp_rot[gp % 3](
                    convT_sb[:, :, gp * P:(gp + 1) * P],
                    mt[:].rearrange("p (ct f) -> p ct f", f=P))
            return run

        def job_A(gp, pool, tag):
            def run():
                for ec in range(ECH):
                    esl = slice(ec * 512, (ec + 1) * 512)
                    mt = pool.tile([P, 512], F32, tag=tag, name=f"amt{gp}_{ec}")
                    for ct in range(GT):
                        nc.tensor.matmul(mt[:],
                                         convT_sb[:, ct, gp * P:(gp + 1) * P],
                                         ow_sb[:, ct, esl],
                                         start=(ct == 0), stop=(ct == GT - 1))
                    cp_rot[(gp + ec + 1) % 3](A_sb[:, gp, esl], mt[:])
            return run

        def job_out(tt, pool, tag, split_store=False):
            def run():
                tsl = slice(tt * P, (tt + 1) * P)
                gts = tt_gts[tt]
                osb = opool.tile([P, D], BF16, tag="osb")
                for ec in range(ECH):
                    esl = slice(ec * 512, (ec + 1) * 512)
                    mt = pool.tile([P, 512], F32, tag=tag, name=f"omt{tt}_{ec}")
                    for i, gt in enumerate(gts):
                        nc.tensor.matmul(mt[:],
                                         ST_sb[:, gt, tsl],
                                         A_sb[:, gt, esl],
                                         start=(i == 0), stop=(i == len(gts) - 1))
                    cp_rot[(tt + ec) % 3](osb[:, esl], mt[:])
                    if split_store:
                        dma_rot[(tt + ec) % 3].dma_start(out_d[tsl, esl],
                                                         osb[:, esl])
                if not split_store:
                    dma_rot[tt % 3].dma_start(out_d[tsl, :], osb[:])
            return run

        # enqueue points: field copy for gt lands during iteration
        # (last_contrib(gt) + 1) via the pending-scatter delay
        last_tt = {gt: max(t for t in range(TT) if gt in tt_gts[t])
                   for gt in range(GT)}
        enqueue_at = {}
        ready2 = last_tt[1] + 2      # field gt0,gt1 copied
        ready3 = last_tt[2] + 2
        enqueue_at.setdefault(ready2, []).append(("cvt", 2))
        enqueue_at.setdefault(ready2 + 1, []).append(("A", 2))
        enqueue_at.setdefault(ready3, []).append(("cvt", 3))
        enqueue_at.setdefault(ready3 + 1, []).append(("A", 3))
        post_loop_outs = []
        for t in range(TT):
            if set(tt_gts[t]) <= {2}:
                enqueue_at.setdefault(ready2 + 2, []).append(("out", t))
            elif set(tt_gts[t]) <= {2, 3}:
                if len(post_loop_outs) < 2 and t >= TT - 3:
                    # held back: fills the PE while the last tile's wv chain
                    # (ACT/DVE) and field copy run before the ph2 tail
                    post_loop_outs.append(t)
                else:
                    enqueue_at.setdefault(ready3 + 2, []).append(("out", t))
        mid_queue = []

        fcopy = {0: nc.vector.tensor_copy, 1: nc.scalar.copy,
                 2: nc.vector.tensor_copy, 3: nc.scalar.copy}

        def emit_scatter(tt, wv):
            for gt, first, last in scatter_plan[tt]:
                if first:
                    field_ps[gt] = ps_f.tile([P, CLOC], F32, tag="fld",
                                             name=f"fld{gt}")
                nc.tensor.matmul(field_ps[gt][:],
                                 S_sb[:, tt, gt * P:(gt + 1) * P], wv[:],
                                 start=first, stop=last)
                if last:
                    fcopy[gt](field_sb[:, gt, :], field_ps[gt][:])

        # x tile management: k-projection runs one tile ahead of v.
        x8_tiles, xb_tiles = {}, {}

        def ensure_x8(tt, eng):
            if tt not in x8_tiles and tt < TT:
                x8_tiles[tt] = xpool.tile([P, KT2, 2, P], FP8, tag="x8blk",
                                          bufs=5, name=f"x8b{tt}")
                eng.dma_start(x8_tiles[tt][:], x8d[:, tt, :])

        def ensure_xb(tt, eng):
            if tt not in xb_tiles and tt < TT:
                xb_tiles[tt] = xpool.tile([P, KT, P], BF16, tag="xbblk",
                                          bufs=5, name=f"xbb{tt}")
                eng.dma_start(xb_tiles[tt][:], xbd[:, tt, :])

        # startup: weight loads batched into halves, ordered by first use
        # across the three DMA queues (SP / ACT / GpSimd).
        ensure_x8(0, nc.sync)
        nc.scalar.dma_start(kw_sb[:, 0:2, :, :], kwd[:, 0:2, :])
        ensure_x8(1, nc.gpsimd)
        ensure_xb(0, nc.sync)
        nc.scalar.dma_start(kw_sb[:, 2:4, :, :], kwd[:, 2:4, :])
        ensure_x8(2, nc.gpsimd)
        nc.scalar.dma_start(vw_sb[:, 0:4, :], vwd[:, 0:4, :])
        nc.sync.dma_start(vw_sb[:, 4:8, :], vwd[:, 4:8, :])
        for tt in range(1, 5):
            ensure_xb(tt, nc.sync)
        for tt in range(3, 5):
            ensure_x8(tt, nc.gpsimd)
        nc.gpsimd.dma_start(S_sb[:, 0:8, :], Sd[:, 0:8, :])

        # bulk constant loads, ordered by first use; ST0/ST1 (only needed in
        # the tail) ride the scalar queue to keep gpsimd free for x8
        gp_loads = {
            2: [(nc.gpsimd, S_sb[:, 8:16, :], Sd[:, 8:16, :])],
            4: [(nc.gpsimd, ST_sb[:, 2, :], STd[:, 2, :])],
            6: [(nc.gpsimd, ST_sb[:, 3, :], STd[:, 3, :])],
            8: [(nc.gpsimd, ct_sb[:], ctd[:]), (nc.gpsimd, ow_sb[:], owd[:])],
            10: [(nc.gpsimd, S_sb[:, 16:24, :], Sd[:, 16:24, :])],
            12: [(nc.scalar, ST_sb[:, 0, :], STd[:, 0, :])],
            14: [(nc.scalar, ST_sb[:, 1, :], STd[:, 1, :])],
            16: [(nc.gpsimd, S_sb[:, 24:32, :], Sd[:, 24:32, :])],
        }

        kps_t = {}
        km_t = {}
        KLAG = 1        # k-projection runs one tile ahead of v

        def emit_k(tt):
            kps_t[tt] = ps_k.tile([P, CLOC], F32, tag="kps", name=f"kps{tt}")
            for kt in range(KT2):
                nc.tensor.matmul(kps_t[tt][:], x8_tiles[tt][:, kt, :, :],
                                 kw_sb[:, kt, :, :],
                                 start=(kt == 0), stop=(kt == KT2 - 1),
                                 perf_mode=DR)

        def emit_km(tt):
            # ||k|| per head right after the k matmuls: frees the kps psum
            # quickly and precomputes km so wv is a single DVE op after v
            # (kps holds 2^7*k; the scale is removed via the conv matrix)
            kps = kps_t.pop(tt)
            if with_kb:
                nc.vector.tensor_tensor(
                    kps[:], kps[:], kb_sb[:].broadcast_to((P, CLOC)),
                    mybir.AluOpType.add)
            ksq = smpool.tile([P, CLOC], F32, tag="ksq", bufs=3,
                              name=f"ksq{tt}")
            nc.scalar.activation(ksq[:], kps[:], mybir.ActivationFunctionType.Square)
            km2 = smpool.tile([P, HLOC], F32, tag="km2", bufs=3,
                              name=f"km2{tt}")
            nc.vector.reduce_sum(km2[:], ksq[:].rearrange("p (h d) -> p h d", d=HD),
                                 axis=mybir.AxisListType.X)
            km_t[tt] = smpool.tile([P, HLOC], F32, tag="km", bufs=KLAG + 3,
                                   name=f"km{tt}")
            nc.scalar.sqrt(km_t[tt][:], km2[:])

        for tt in range(KLAG):      # prologue: pure-k warmup
            emit_k(tt)
            emit_km(tt)
        pending = None
        for tt in range(TT):
            # alternate x8 issues between the two pure-DMA queues so they
            # never pile up behind a 1MB constant chunk on gpsimd (measured
            # k-proj LDWEIGHTS waits of 3.4-5us on x8 sems at iters 9-11)
            ensure_x8(tt + KLAG + 3, nc.sync if tt % 2 else nc.gpsimd)
            ensure_xb(tt + 3, nc.sync)
            for eng, dst, src in gp_loads.get(tt, []):
                eng.dma_start(dst, src)

            if tt + KLAG < TT:
                emit_k(tt + KLAG)
                emit_km(tt + KLAG)
            vps = ps_v.tile([P, CLOC], F32, tag="vps")
            korder = [4, 5, 6, 7, 0, 1, 2, 3] if tt < 2 else list(range(KT))
            for i, kt in enumerate(korder):
                nc.tensor.matmul(vps[:], xb_tiles[tt][:, kt, :], vw_sb[:, kt, :],
                                 start=(i == 0), stop=(i == KT - 1))
            if pending is not None:
                emit_scatter(*pending)
                pending = None
            if with_vb:
                nc.vector.tensor_tensor(
                    vps[:], vps[:], vb_sb[:].broadcast_to((P, CLOC)),
                    mybir.AluOpType.add)

            for kind, arg in enqueue_at.get(tt, []):
                mid_queue.append((kind, arg))
            for _ in range(2):
                if mid_queue:
                    kind, arg = mid_queue.pop(0)
                    mk = {"cvt": job_convT, "A": job_A, "out": job_out}[kind]
                    mk(arg, ps_mid, "mid")()

            # wv = v * ||k||, one DVE op via stride-0 broadcast of km
            km = km_t.pop(tt)
            wv = wvpool.tile([P, CLOC], BF16, tag="wv")
            nc.vector.tensor_tensor(
                wv[:].rearrange("p (h d) -> p h d", d=HD),
                vps[:].rearrange("p (h d) -> p h d", d=HD),
                km[:].unsqueeze(2).broadcast_to((P, HLOC, HD)),
                mybir.AluOpType.mult)
            pending = (tt, wv)

        emit_scatter(*pending)
        for t in post_loop_outs:
            job_out(t, ps_mid, "mid")()
        ph1.close()

        # flush any queued mid jobs (still inside ph1 pools)
        while mid_queue:
            kind, arg = mid_queue.pop(0)
            mk = {"cvt": job_convT, "A": job_A, "out": job_out}[kind]
            mk(arg, ps_mid, "mid")()

        # ---- tail: circular-wrap convT tiles 0,1 -> A -> remaining tokens ----
        ph2 = ExitStack()
        ps_t = ph2.enter_context(tc.tile_pool(name="ps_t", bufs=8, space="PSUM"))
        done = {t for jobs in enqueue_at.values() for k, t in jobs if k == "out"}
        done |= set(post_loop_outs)
        job_convT(0, ps_t, "tmid")()
        job_convT(1, ps_t, "tmid")()
        job_A(0, ps_t, "tmid")()
        job_A(1, ps_t, "tmid")()
        for t in range(TT):
            if t not in done:
                job_out(t, ps_t, "tmid")()
        ph2.close()

    nc.compile()
    return nc


_PROGRAM_CACHE = {}


def _get_program(with_kb, with_vb):
    key = (with_kb, with_vb)
    if key not in _PROGRAM_CACHE:
        _, _, _, _, sp, tg, cb = _plans()
        _PROGRAM_CACHE[key] = _build_program(with_kb, with_vb, sp, tg, cb)
    return _PROGRAM_CACHE[key]


def kernel(x, q_w, q_b, k_w, k_b, v_w, v_b, out_w, out_b):
    global LAST_RESULT
    x = np.asarray(x, dtype=np.float32)
    k_w = np.asarray(k_w, dtype=np.float32)
    k_b = np.asarray(k_b, dtype=np.float32)
    v_w = np.asarray(v_w, dtype=np.float32)
    v_b = np.asarray(v_b, dtype=np.float32)
    out_w = np.asarray(out_w, dtype=np.float32)
    out_b = np.asarray(out_b, dtype=np.float32)

    with_kb = bool(np.any(k_b))
    with_vb = bool(np.any(v_b))
    nc = _get_program(with_kb, with_vb)
    _, CTm, Smat, STm, _, _, _ = _plans()

    S_host = np.ascontiguousarray(
        Smat.reshape(TT, P, G).transpose(1, 0, 2)).astype(NP_BF16)
    ST_host = np.ascontiguousarray(
        STm.reshape(GT, P, N).transpose(1, 0, 2)).astype(NP_BF16)
    ct_host = np.ascontiguousarray(
        (CTm / WSCALE).astype(NP_BF16).reshape(GT, P, G).transpose(1, 0, 2))

    # per-batch x layouts (shared by both head-group cores of the batch)
    x8_host, xb_host = [], []
    for b in range(B):
        xt = x[b]                                   # (N, D)
        a8 = xt.reshape(TT, P, KT2, 2, P).transpose(4, 0, 2, 3, 1)
        x8_host.append(np.ascontiguousarray(a8).astype(NP_FP8).reshape(P, TT, KT2 * 2 * P))
        ab = xt.reshape(TT, P, KT, P).transpose(3, 0, 2, 1)
        xb_host.append(np.ascontiguousarray(ab).astype(NP_BF16).reshape(P, TT, KT * P))

    in_maps = []
    for c in range(NCORES):
        b, hg = c // 2, c % 2
        chs = slice(hg * CLOC, (hg + 1) * CLOC)
        kwT = np.ascontiguousarray(k_w[chs, :].T) * np.float32(WSCALE)  # (D, CLOC)
        vwT = np.ascontiguousarray(v_w[chs, :].T)
        owT = np.ascontiguousarray(out_w[:, chs].T)                     # (CLOC, D)
        m = {
            "x8": x8_host[b],
            "xb": xb_host[b],
            "kw8": np.ascontiguousarray(
                kwT.reshape(KT2, 2, P, CLOC).transpose(2, 0, 1, 3)
            ).astype(NP_FP8).reshape(P, KT2, 2 * CLOC),
            "vw": np.ascontiguousarray(
                vwT.reshape(KT, P, CLOC).transpose(1, 0, 2)).astype(NP_BF16),
            "ow": np.ascontiguousarray(
                owT.reshape(GT, P, D).transpose(1, 0, 2)).astype(NP_BF16),
            "ct": ct_host,
            "Smat": S_host,
            "STm": ST_host,
        }
        if with_kb:
            m["kb"] = np.ascontiguousarray(
                k_b[chs][None, :] * np.float32(WSCALE)).astype(np.float32)
        if with_vb:
            m["vb"] = np.ascontiguousarray(v_b[chs][None, :]).astype(np.float32)
        in_maps.append(m)

    res = run_bass_kernel_spmd(nc, in_maps, core_ids=list(range(NCORES)),
                               trace=TRACE)
    LAST_RESULT = res

    out = np.empty((B, N, D), dtype=np.float32)
    for b in range(B):
        out[b] = res.results[2 * b]["out"].astype(np.float32)
        out[b] += res.results[2 * b + 1]["out"].astype(np.float32)
        out[b] += out_b[None, :]
    return out
